# revision 34
# baseline (speedup 1.0000x reference)
"""Trainium2 Bass kernel for nn_BinaryFCNN (4-layer binary MLP).

Math per layer: y = sign(x @ sign(W).T + b), sign(0) -> +1.
Shapes: x [8192, 4096], W [4096, 4096] x4, b [4096] x4.

Strategy: data-parallel over batch across 8 NeuronCores (weights
replicated, streamed from DRAM). Activations live feature-major in SBUF
([feature, batch]) so every layer's matmul contracts over the partition
dim with weight tiles stationary; no transposes on chip.

Numerics: layers 2-4 have +-1 inputs and +-1 weights, so PSUM fp32 sums
are exact integers -> bit-exact regardless of order. All fp error comes
from layer 1, which is computed as x = hi + lo (two fp16 operands, both
products exact because weights are +-1) accumulated in fp32 PSUM --
error ~1e-5, the same scale as any fp32 matmul's own rounding.

Host side does layout only: transpose/tile/dtype-cast of weights and x,
sharding, and the inverse layout on the output. All value computation
(sign of weights, matmuls, bias, sign activations) happens on-chip.
"""

import numpy as np
import ml_dtypes

import concourse.mybir as mybir
import concourse.tile as tile
from concourse import bacc
from concourse.bass_utils import run_bass_kernel_spmd

F32 = mybir.dt.float32
F16 = mybir.dt.float16
BF16 = mybir.dt.bfloat16
F8 = mybir.dt.float8e4
SIGN = mybir.ActivationFunctionType.Sign

N_CORES = 8
BATCH = 8192
D = 4096
B = BATCH // N_CORES      # 1024 rows per core
KC = D // 128             # 32 contraction chunks
OT = D // 128             # 32 output tiles
BH = 512                  # layer-1 batch half (bf16 fallback mode)
NBT = B // 512            # 2 b-tiles of 512 for layers 2-4

# L1_MODE: "f16x2" = hi/lo fp16 split (2 full-rate passes);
#          "f32"   = native fp32 matmul (4 cycles/row, ~2x slower L1)
L1_MODE = "f16x2"
# L234_MODE: "bf16" = bf16 matmuls; "fp8dr" = fp8e4m3 + DoubleRow
# (2 k-chunks per matmul; +-1 values exact in fp8, sums still exact ints)
L234_MODE = "fp8dr"


def _build(l1_mode=L1_MODE, l234_mode=L234_MODE):
    nc = bacc.Bacc("TRN2", target_bir_lowering=False, debug=False,
                   num_devices=N_CORES)
    adt = F8 if l234_mode == "fp8dr" else BF16
    xt = nc.declare_dram_parameter("xt", [D, B], F32, isOutput=False)
    wps = [nc.declare_dram_parameter(f"w{l}", [OT, 128, KC, 128], BF16,
                                     isOutput=False) for l in range(4)]
    bp = nc.declare_dram_parameter("ball", [128, 4, OT], F32, isOutput=False)
    out = nc.declare_dram_parameter("out", [OT, 128, B], adt, isOutput=True)

    xt_r = xt.rearrange("(kc p) b -> p kc b", p=128)

    fp8 = l234_mode == "fp8dr"
    with tile.TileContext(nc) as tc:
        with (
            tc.tile_pool(name="big", bufs=2) as big,
            tc.tile_pool(name="xhl", bufs=2) as xhlp,
            tc.tile_pool(name="xstage", bufs=2) as xstage,
            tc.tile_pool(name="wraw", bufs=2) as wrawp,
            tc.tile_pool(name="wsgn", bufs=2) as wsgnp,
            tc.tile_pool(name="bias", bufs=1) as biasp,
            tc.tile_pool(name="psum", bufs=4, space="PSUM") as psump,
        ):
            ball = biasp.tile([128, 4, OT], F32, tag="ball")
            nc.sync.dma_start(ball[:], bp[:])
            bias_t = [ball[:, l, :] for l in range(4)]
            # epsilon bias for weight Sign: rescues exact-zero weights to
            # +1 (reference maps sign(0) -> +1); min nonzero |w| is ~9e-10.
            eps_t = biasp.tile([128, 1], F32, tag="eps")
            nc.any.memset(eps_t[:], 1e-15)

            l1_f16 = l1_mode == "f16x2"

            # ---------------- layer 1 ----------------
            # fp8 A-chain frees enough SBUF for rotating hi/lo buffers:
            # 4 quarter-passes of 256 batch cols, double-buffered, so the
            # hi/lo conversion of pass h+1 overlaps pass h's matmuls.
            PB = 256 if fp8 else BH
            w1dt = F16 if l1_f16 else F32
            if not fp8:
                # bf16 A-chain leaves no room for rotation: one shared
                # buffer in the big pool, slot0 (later reused by A2/Y4);
                # must be allocated BEFORE a1 so a1 lands in slot1.
                xhl_shape = ([128, 2, KC, PB], F16) if l1_f16 \
                    else ([128, KC, PB], F32)
                xhl_shared = big.tile(*xhl_shape, tag="abuf")
            a1 = big.tile([128, OT, B], adt, tag="abuf")
            for h in range(B // PB):
                boff = h * PB
                if fp8:
                    if l1_f16:
                        xhl = xhlp.tile([128, 2, KC, PB], F16, tag="xhl")
                    else:
                        xhl = xhlp.tile([128, KC, PB], F32, tag="xhl")
                else:
                    xhl = xhl_shared
                # k-major conversion steps: the first matmul group reads
                # chunks in kc order, so it can start after step 0; also
                # gives 1KB-contiguous DMA runs instead of 256B.
                KSTEP = 8

                def conv_step(q, xhl=xhl, boff=boff):
                    ks = slice(q * KSTEP, (q + 1) * KSTEP)
                    src = xt_r[:, ks, boff:boff + PB]
                    if l1_f16:
                        xs = xstage.tile([128, KSTEP, PB], F32, tag="xs")
                        nc.sync.dma_start(xs[:], src)
                        nc.vector.tensor_copy(out=xhl[:, 0, ks, :], in_=xs[:])
                        nc.vector.tensor_tensor(
                            xhl[:, 1, ks, :], xs[:], xhl[:, 0, ks, :],
                            mybir.AluOpType.subtract)
                    else:
                        nc.sync.dma_start(xhl[:, ks, :], src)

                conv_step(0)
                if h == 0:
                    # first weight tile right after the first conversion
                    # DMA on the sync ring: its Sign runs on ACT in
                    # parallel with the DVE hi/lo of step 0, so neither
                    # chain blocks the first matmul group
                    wr = wrawp.tile([128, KC, 128], BF16, tag="wr")
                    nc.sync.dma_start(wr[:], wps[0][0])
                    pre_ws = wsgnp.tile([128, KC, 128], w1dt, tag="ws")
                    nc.scalar.activation(pre_ws[:], wr[:], SIGN,
                                         bias=eps_t[:])
                for q in range(1, KC // KSTEP):
                    conv_step(q)
                for ot in range(OT):
                    if h == 0 and ot == 0:
                        ws = pre_ws
                    else:
                        wr = wrawp.tile([128, KC, 128], BF16, tag="wr")
                        nc.sync.dma_start(wr[:], wps[0][ot])
                        ws = wsgnp.tile([128, KC, 128], w1dt, tag="ws")
                        nc.scalar.activation(ws[:], wr[:], SIGN,
                                             bias=eps_t[:])
                    ps = psump.tile([128, PB], F32, tag="ps")
                    if l1_f16:
                        for kc in range(KC):
                            nc.tensor.matmul(ps[:], ws[:, kc, :],
                                             xhl[:, 0, kc, :],
                                             start=(kc == 0), stop=False)
                            nc.tensor.matmul(ps[:], ws[:, kc, :],
                                             xhl[:, 1, kc, :],
                                             start=False, stop=(kc == KC - 1))
                    else:
                        for kc in range(KC):
                            nc.tensor.matmul(ps[:], ws[:, kc, :],
                                             xhl[:, kc, :],
                                             start=(kc == 0),
                                             stop=(kc == KC - 1))
                    nc.scalar.activation(a1[:, ot, boff:boff + PB], ps[:],
                                         SIGN, bias=bias_t[0][:, ot:ot + 1])

            # ---------------- layers 2..4 ----------------
            dr = mybir.MatmulPerfMode.DoubleRow if fp8 else None
            ain = a1
            for l in range(1, 4):
                aout = big.tile([128, OT, B], adt, tag="abuf")
                for ot in range(OT):
                    wr = wrawp.tile([128, KC, 128], BF16, tag="wr")
                    nc.sync.dma_start(wr[:], wps[l][ot])
                    ws = wsgnp.tile([128, KC, 128], adt, tag="ws")
                    nc.scalar.activation(ws[:], wr[:], SIGN, bias=eps_t[:])
                    for bt in range(NBT):
                        bs = slice(bt * 512, (bt + 1) * 512)
                        ps = psump.tile([128, 512], F32, tag="ps")
                        if fp8:
                            for k2 in range(KC // 2):
                                ksl = slice(2 * k2, 2 * k2 + 2)
                                nc.tensor.matmul(ps[:], ws[:, ksl, :],
                                                 ain[:, ksl, bs],
                                                 start=(k2 == 0),
                                                 stop=(k2 == KC // 2 - 1),
                                                 perf_mode=dr)
                        else:
                            for kc in range(KC):
                                nc.tensor.matmul(ps[:], ws[:, kc, :],
                                                 ain[:, kc, bs],
                                                 start=(kc == 0),
                                                 stop=(kc == KC - 1))
                        nc.scalar.activation(aout[:, ot, bs], ps[:], SIGN,
                                             bias=bias_t[l][:, ot:ot + 1])
                        if l == 3:
                            # store per half-tile so the last transfer
                            # starts as early as possible
                            nc.sync.dma_start(out[ot][:, bs],
                                              aout[:, ot, bs])
                ain = aout
    nc.compile()
    return nc


_NC_CACHE = {}


def _get_nc(l1_mode=L1_MODE, l234_mode=L234_MODE):
    key = (l1_mode, l234_mode)
    if key not in _NC_CACHE:
        _NC_CACHE[key] = _build(l1_mode, l234_mode)
    return _NC_CACHE[key]


def _prep_weights(W):
    # [ot, p, kc, o] with arr[ot, p, kc, o] = W[ot*128 + o, kc*128 + p]
    t = np.asarray(W, dtype=np.float32).reshape(OT, 128, KC, 128)
    t = np.ascontiguousarray(t.transpose(0, 3, 2, 1))
    return t.astype(ml_dtypes.bfloat16)


def _prep_bias(b):
    return np.ascontiguousarray(np.asarray(b, np.float32).reshape(OT, 128).T)


def kernel(x, W0, b0, W1, b1, W2, b2, W3, b3, l1_mode=L1_MODE,
           l234_mode=L234_MODE, _results=None):
    nc = _get_nc(l1_mode, l234_mode)
    x = np.asarray(x, dtype=np.float32)
    shared = {}
    for l, W in enumerate((W0, W1, W2, W3)):
        shared[f"w{l}"] = _prep_weights(W)
    shared["ball"] = np.ascontiguousarray(
        np.stack([_prep_bias(b) for b in (b0, b1, b2, b3)], axis=1))
    in_maps = []
    for c in range(N_CORES):
        m = dict(shared)
        m["xt"] = np.ascontiguousarray(x[c * B:(c + 1) * B].T)
        in_maps.append(m)
    res = run_bass_kernel_spmd(nc, in_maps, core_ids=list(range(N_CORES)))
    if _results is not None:
        _results.append(res)
    y = np.empty((BATCH, D), dtype=np.float32)
    for c in range(N_CORES):
        o = np.asarray(res.results[c]["out"], dtype=np.float32)
        y[c * B:(c + 1) * B] = o.transpose(2, 0, 1).reshape(B, D)
    return y


# revision 36
# speedup vs baseline: 1.0325x; 1.0325x over previous
"""Trainium2 Bass kernel for nn_BinaryFCNN (4-layer binary MLP).

Math per layer: y = sign(x @ sign(W).T + b), sign(0) -> +1.
Shapes: x [8192, 4096], W [4096, 4096] x4, b [4096] x4.

Strategy: data-parallel over batch across 8 NeuronCores (weights
replicated, streamed from DRAM). Activations live feature-major in SBUF
([feature, batch]) so every layer's matmul contracts over the partition
dim with weight tiles stationary; no transposes on chip.

Numerics: layers 2-4 have +-1 inputs and +-1 weights, so PSUM fp32 sums
are exact integers -> bit-exact regardless of order. All fp error comes
from layer 1, which is computed as x = hi + lo (two fp16 operands, both
products exact because weights are +-1) accumulated in fp32 PSUM --
error ~1e-5, the same scale as any fp32 matmul's own rounding.

Host side does layout only: transpose/tile/dtype-cast of weights and x,
sharding, and the inverse layout on the output. All value computation
(sign of weights, matmuls, bias, sign activations) happens on-chip.
"""

import numpy as np
import ml_dtypes

import concourse.mybir as mybir
import concourse.tile as tile
from concourse import bacc
from concourse.bass_utils import run_bass_kernel_spmd

F32 = mybir.dt.float32
F16 = mybir.dt.float16
BF16 = mybir.dt.bfloat16
F8 = mybir.dt.float8e4
SIGN = mybir.ActivationFunctionType.Sign

N_CORES = 8
BATCH = 8192
D = 4096
B = BATCH // N_CORES      # 1024 rows per core
KC = D // 128             # 32 contraction chunks
OT = D // 128             # 32 output tiles
BH = 512                  # layer-1 batch half (bf16 fallback mode)
NBT = B // 512            # 2 b-tiles of 512 for layers 2-4

# L1_MODE: "f16x2" = hi/lo fp16 split (2 full-rate passes);
#          "f32"   = native fp32 matmul (4 cycles/row, ~2x slower L1)
L1_MODE = "f16x2"
# L234_MODE: "bf16" = bf16 matmuls; "fp8dr" = fp8e4m3 + DoubleRow
# (2 k-chunks per matmul; +-1 values exact in fp8, sums still exact ints)
L234_MODE = "fp8dr"


def _build(l1_mode=L1_MODE, l234_mode=L234_MODE):
    nc = bacc.Bacc("TRN2", target_bir_lowering=False, debug=False,
                   num_devices=N_CORES)
    adt = F8 if l234_mode == "fp8dr" else BF16
    xt = nc.declare_dram_parameter("xt", [D, B], F32, isOutput=False)
    wps = [nc.declare_dram_parameter(f"w{l}", [OT, 128, KC, 128], BF16,
                                     isOutput=False) for l in range(4)]
    bp = nc.declare_dram_parameter("ball", [128, 4, OT], F32, isOutput=False)
    out = nc.declare_dram_parameter("out", [OT, 128, B], adt, isOutput=True)

    xt_r = xt.rearrange("(kc p) b -> p kc b", p=128)

    fp8 = l234_mode == "fp8dr"
    with tile.TileContext(nc) as tc:
        with (
            tc.tile_pool(name="big", bufs=2) as big,
            tc.tile_pool(name="xhl", bufs=2) as xhlp,
            tc.tile_pool(name="xstage", bufs=2) as xstage,
            tc.tile_pool(name="wraw", bufs=2) as wrawp,
            tc.tile_pool(name="wsgn", bufs=3) as wsgnp,
            tc.tile_pool(name="bias", bufs=1) as biasp,
            tc.tile_pool(name="psum", bufs=6, space="PSUM") as psump,
        ):
            ball = biasp.tile([128, 4, OT], F32, tag="ball")
            nc.sync.dma_start(ball[:], bp[:])
            bias_t = [ball[:, l, :] for l in range(4)]
            # epsilon bias for weight Sign: rescues exact-zero weights to
            # +1 (reference maps sign(0) -> +1); min nonzero |w| is ~9e-10.
            eps_t = biasp.tile([128, 1], F32, tag="eps")
            nc.any.memset(eps_t[:], 1e-15)

            l1_f16 = l1_mode == "f16x2"

            # ---------------- layer 1 ----------------
            # fp8 A-chain frees enough SBUF for rotating hi/lo buffers:
            # 4 quarter-passes of 256 batch cols, double-buffered, so the
            # hi/lo conversion of pass h+1 overlaps pass h's matmuls.
            PB = 256 if fp8 else BH
            w1dt = F16 if l1_f16 else F32
            if not fp8:
                # bf16 A-chain leaves no room for rotation: one shared
                # buffer in the big pool, slot0 (later reused by A2/Y4);
                # must be allocated BEFORE a1 so a1 lands in slot1.
                xhl_shape = ([128, 2, KC, PB], F16) if l1_f16 \
                    else ([128, KC, PB], F32)
                xhl_shared = big.tile(*xhl_shape, tag="abuf")
            a1 = big.tile([128, OT, B], adt, tag="abuf")
            for h in range(B // PB):
                boff = h * PB
                if fp8:
                    if l1_f16:
                        xhl = xhlp.tile([128, 2, KC, PB], F16, tag="xhl")
                    else:
                        xhl = xhlp.tile([128, KC, PB], F32, tag="xhl")
                else:
                    xhl = xhl_shared
                if h == 0:
                    # first weight tile ahead of the 1MB conversion DMAs
                    # on the sync ring, so the first matmul group isn't
                    # blocked on a queued-late weight load
                    wr = wrawp.tile([128, KC, 128], BF16, tag="wr")
                    nc.sync.dma_start(wr[:], wps[0][0])
                    pre_ws = wsgnp.tile([128, KC, 128], w1dt, tag="ws")
                    nc.scalar.activation(pre_ws[:], wr[:], SIGN,
                                         bias=eps_t[:])
                # k-major conversion steps: the first matmul group reads
                # chunks in kc order, so it can start after step 0; also
                # gives 1KB-contiguous DMA runs instead of 256B.
                KSTEP = 8
                for q in range(KC // KSTEP):
                    ks = slice(q * KSTEP, (q + 1) * KSTEP)
                    src = xt_r[:, ks, boff:boff + PB]
                    if l1_f16:
                        xs = xstage.tile([128, KSTEP, PB], F32, tag="xs")
                        nc.sync.dma_start(xs[:], src)
                        nc.vector.tensor_copy(out=xhl[:, 0, ks, :], in_=xs[:])
                        nc.vector.tensor_tensor(
                            xhl[:, 1, ks, :], xs[:], xhl[:, 0, ks, :],
                            mybir.AluOpType.subtract)
                    else:
                        nc.sync.dma_start(xhl[:, ks, :], src)
                for ot in range(OT):
                    if h == 0 and ot == 0:
                        ws = pre_ws
                    else:
                        wr = wrawp.tile([128, KC, 128], BF16, tag="wr")
                        nc.sync.dma_start(wr[:], wps[0][ot])
                        ws = wsgnp.tile([128, KC, 128], w1dt, tag="ws")
                        nc.scalar.activation(ws[:], wr[:], SIGN,
                                             bias=eps_t[:])
                    ps = psump.tile([128, PB], F32, tag="ps")
                    if l1_f16:
                        for kc in range(KC):
                            nc.tensor.matmul(ps[:], ws[:, kc, :],
                                             xhl[:, 0, kc, :],
                                             start=(kc == 0), stop=False)
                            nc.tensor.matmul(ps[:], ws[:, kc, :],
                                             xhl[:, 1, kc, :],
                                             start=False, stop=(kc == KC - 1))
                    else:
                        for kc in range(KC):
                            nc.tensor.matmul(ps[:], ws[:, kc, :],
                                             xhl[:, kc, :],
                                             start=(kc == 0),
                                             stop=(kc == KC - 1))
                    nc.scalar.activation(a1[:, ot, boff:boff + PB], ps[:],
                                         SIGN, bias=bias_t[0][:, ot:ot + 1])

            # ---------------- layers 2..4 ----------------
            dr = mybir.MatmulPerfMode.DoubleRow if fp8 else None
            ain = a1
            for l in range(1, 4):
                aout = big.tile([128, OT, B], adt, tag="abuf")
                for ot in range(OT):
                    wr = wrawp.tile([128, KC, 128], BF16, tag="wr")
                    nc.sync.dma_start(wr[:], wps[l][ot])
                    ws = wsgnp.tile([128, KC, 128], adt, tag="ws")
                    nc.scalar.activation(ws[:], wr[:], SIGN, bias=eps_t[:])
                    for bt in range(NBT):
                        bs = slice(bt * 512, (bt + 1) * 512)
                        ps = psump.tile([128, 512], F32, tag="ps")
                        if fp8:
                            for k2 in range(KC // 2):
                                ksl = slice(2 * k2, 2 * k2 + 2)
                                nc.tensor.matmul(ps[:], ws[:, ksl, :],
                                                 ain[:, ksl, bs],
                                                 start=(k2 == 0),
                                                 stop=(k2 == KC // 2 - 1),
                                                 perf_mode=dr)
                        else:
                            for kc in range(KC):
                                nc.tensor.matmul(ps[:], ws[:, kc, :],
                                                 ain[:, kc, bs],
                                                 start=(kc == 0),
                                                 stop=(kc == KC - 1))
                        nc.scalar.activation(aout[:, ot, bs], ps[:], SIGN,
                                             bias=bias_t[l][:, ot:ot + 1])
                        if l == 3:
                            # store per half-tile so the last transfer
                            # starts as early as possible
                            nc.sync.dma_start(out[ot][:, bs],
                                              aout[:, ot, bs])
                ain = aout
    nc.compile()
    return nc


_NC_CACHE = {}


def _get_nc(l1_mode=L1_MODE, l234_mode=L234_MODE):
    key = (l1_mode, l234_mode)
    if key not in _NC_CACHE:
        _NC_CACHE[key] = _build(l1_mode, l234_mode)
    return _NC_CACHE[key]


def _prep_weights(W):
    # [ot, p, kc, o] with arr[ot, p, kc, o] = W[ot*128 + o, kc*128 + p]
    t = np.asarray(W, dtype=np.float32).reshape(OT, 128, KC, 128)
    t = np.ascontiguousarray(t.transpose(0, 3, 2, 1))
    return t.astype(ml_dtypes.bfloat16)


def _prep_bias(b):
    return np.ascontiguousarray(np.asarray(b, np.float32).reshape(OT, 128).T)


def kernel(x, W0, b0, W1, b1, W2, b2, W3, b3, l1_mode=L1_MODE,
           l234_mode=L234_MODE, _results=None):
    nc = _get_nc(l1_mode, l234_mode)
    x = np.asarray(x, dtype=np.float32)
    shared = {}
    for l, W in enumerate((W0, W1, W2, W3)):
        shared[f"w{l}"] = _prep_weights(W)
    shared["ball"] = np.ascontiguousarray(
        np.stack([_prep_bias(b) for b in (b0, b1, b2, b3)], axis=1))
    in_maps = []
    for c in range(N_CORES):
        m = dict(shared)
        m["xt"] = np.ascontiguousarray(x[c * B:(c + 1) * B].T)
        in_maps.append(m)
    res = run_bass_kernel_spmd(nc, in_maps, core_ids=list(range(N_CORES)))
    if _results is not None:
        _results.append(res)
    y = np.empty((BATCH, D), dtype=np.float32)
    for c in range(N_CORES):
        o = np.asarray(res.results[c]["out"], dtype=np.float32)
        y[c * B:(c + 1) * B] = o.transpose(2, 0, 1).reshape(B, D)
    return y
